# revision 43
# baseline (speedup 1.0000x reference)
"""Trainium2 Bass kernel for nn_HNC_strategy (hypernetwork-conditioned MLP).

Math (per sample b):
  A[b,:]   = tanh-MLP hypernet of [t-0.5, freqs[b]]          -> [8]
  params   = A @ head_w.T + head_b                           -> [P] (never materialized)
  x[b,:]   = [cos(y[b,:64]), sin(y[b,:64])]                  -> [128]
  hid      = tanh(W1[b] @ x[b] + b1[b])                      -> [65]
  out      = W2[b] @ hid + b2[b]                             -> [64]

Key identity: W1[b] = sum_k A[b,k] * w1[:,:,k] (+ head_b part), so
  W1[b] @ x[b] = sum_k A[b,k] * (x[b] @ w1k^T)
over 8 shared matmuls G_k = x @ w1k^T. Same for layer 2. The per-sample
k-contraction runs on the vector engine: one multiply per 128-row tile
(G in PSUM x A broadcast), then binary-tree adds batched over a whole
512-row chunk (4 tiles per op) to amortize DVE op overheads. Bias terms
(b1/b2 from A @ head_w-slices) and the 65th hidden unit are also applied
chunk-batched.

All matmuls/transposes run in bf16 (fp32 is 4 cyc/row on the PE; bf16 is 1)
with fp32 PSUM accumulation.

Sharding: pure data parallel over 8 NeuronCores (2048 rows each).
"""

import sys

sys.path.insert(0, "/opt/trn_rl_repo")

import numpy as np
import ml_dtypes

import concourse.bacc as bacc
import concourse.mybir as mybir
import concourse.tile as tile
from concourse.alu_op_type import AluOpType
from concourse.bass_utils import run_bass_kernel_spmd


def _register_mul_cumsum():
    """Runtime-register a custom DVE op: out = cumsum(in0 * in1) along the
    free axis. With (h,k)-inner G layout, per-sample k-contractions become
    one strided subtract of the running sums at segment boundaries."""
    import numpy as np_
    import concourse.dve_ops as dve_ops
    from concourse.dve_ops import DveOp
    from concourse.dve_spec import Spec, Src0, Src1, AluOp, scan, lower
    from concourse.dve_uop import DveOpSpec

    name = "MUL_CUMSUM_ANT"
    if name in dve_ops._SUB_OPCODE_FOR_NAME:
        return next(op for op in dve_ops.OPS if op.name == name)
    spec = Spec(body=scan(AluOp.ADD, Src0 * Src1),
                reference=lambda in0, in1, s0, s1, imm2:
                    np_.cumsum(in0 * in1, axis=-1))
    shas = {}
    for ver in ("v3", "v4"):
        try:
            sp = DveOpSpec(name=name, opcode=1,
                           uops=lower(spec, ver=ver), rd1_en=True)
            shas[ver] = sp.sha(ver)
        except Exception:
            pass
    op = DveOp(name, spec, subdim=False, uops_sha=shas)
    dve_ops._SUB_OPCODE_FOR_NAME[name] = (dve_ops._CUSTOM_DVE_ROW_BASE
                                          + len(dve_ops.OPS))
    dve_ops.OPS.append(op)
    dve_ops.CUSTOM_DVE_SPECS[name] = op.spec
    return op


MULSCAN = _register_mul_cumsum()

DIM = 64
MLPS = DIM + 1          # 65
B = 16384
H = DIM + 2             # 66
P = MLPS * 2 * DIM + MLPS + DIM * MLPS + DIM
O1 = MLPS * 2 * DIM     # 8320  end of W1 block
O2 = O1 + MLPS          # 8385  end of b1 block
O3 = O2 + DIM * MLPS    # 12545 end of W2 block
N_CORES = 8
BS = B // N_CORES       # 2048 rows per core
CH = 512                # chunk = 4 tiles of 128 rows
NCH = BS // CH          # 4
NT = BS // 128          # 16 batch tiles per core

F32 = mybir.dt.float32
BF16 = mybir.dt.bfloat16
TANH = mybir.ActivationFunctionType.Tanh
BF16NP = ml_dtypes.bfloat16

# wpack layout (free-dim offsets of the packed small-weight tile, bf16)
#   w0T  [MLPS,H]  @ 0        w1T [H,H] @ 66     w2T [H,H] @ 132
#   w3T  [H,8]     @ 198      hb1w [9,MLPS] @ 206  w3re [9,DIM] @ 271
#   w1x  [2D,8]    @ 335      ident [128,128] @ 343
WO_W0T, WO_W1T, WO_W2T = 0, 66, 132
WO_W3T, WO_HB1W, WO_W3RE = 198, 206, 271
WO_W1X, WO_ID = 335, 343
WPACK_W = 343 + 128     # 471

_CACHE: dict = {}


def build_bass(has_hbw1: bool, has_hbw2: bool, has_hb3: bool):
    nc = bacc.Bacc("TRN2", target_bir_lowering=False, debug=False,
                   num_devices=N_CORES)

    di = lambda name, shape, dt=BF16: nc.dram_tensor(name, shape, dt,
                                                     kind="ExternalInput")
    d_zT = di("zT", [MLPS, BS])          # [t-0.5 ; freqs^T] per-core shard
    d_xT = di("xT", [2 * DIM, BS])       # [cos ; sin] host-computed, bf16
    d_wpack = di("wpack", [128, WPACK_W])
    d_bpack = di("bpack", [H, 4], F32)   # hb0|hb1|hb2|hb3(pad)
    d_w1r = di("w1r", [2 * DIM, 8 * DIM])    # [i, k*64+h] = head_w[h*128+i, k], h<64
    d_w2r = di("w2r", [MLPS, 8 * DIM])       # [h, k*64+o] = head_w[O2+o*65+h, k]
    d_hbw1 = di("hbw1T", [2 * DIM, MLPS]) if has_hbw1 else None
    d_hbw2 = di("hbw2T", [MLPS, DIM]) if has_hbw2 else None
    d_hb3r = di("hb3r", [128, 8], F32) if has_hb3 else None
    d_ones = di("ones", [1, BS])
    d_out = nc.dram_tensor("out", [BS, DIM], F32, kind="ExternalOutput")

    mult, add = AluOpType.mult, AluOpType.add
    PSUM = "PSUM"

    with tile.TileContext(nc) as tc:
        with tc.tile_pool(name="const", bufs=1) as cp:
            wpack = cp.tile([128, WPACK_W], BF16)
            bpack = cp.tile([H, 4], F32)
            w1r = cp.tile([2 * DIM, 8 * DIM], BF16)
            w2r = cp.tile([MLPS, 8 * DIM], BF16)
            # small weights as one packed DMA on gpsimd; the big reshaped
            # head weights go on sync after chunk-0 inputs (see below)
            nc.gpsimd.dma_start(wpack[:], d_wpack[:])
            nc.gpsimd.dma_start(bpack[:], d_bpack[:])
            w0T = wpack[0:MLPS, WO_W0T:WO_W0T + H]
            w1T = wpack[0:H, WO_W1T:WO_W1T + H]
            w2T = wpack[0:H, WO_W2T:WO_W2T + H]
            w3T = wpack[0:H, WO_W3T:WO_W3T + 8]
            hb1w = wpack[0:9, WO_HB1W:WO_HB1W + MLPS]
            w3re = wpack[0:9, WO_W3RE:WO_W3RE + DIM]
            w1x = wpack[0:2 * DIM, WO_W1X:WO_W1X + 8]
            ident = wpack[0:128, WO_ID:WO_ID + 128]
            hb0 = bpack[0:H, 0:1]
            hb1 = bpack[0:H, 1:2]
            hb2 = bpack[0:H, 2:3]
            hb3 = bpack[0:8, 3:4]
            hbw1 = hbw2 = hb3r = None
            if has_hbw1:
                hbw1 = cp.tile([2 * DIM, MLPS], BF16)
                nc.scalar.dma_start(hbw1[:], d_hbw1[:])
            if has_hbw2:
                hbw2 = cp.tile([MLPS, DIM], BF16)
                nc.scalar.dma_start(hbw2[:], d_hbw2[:])
            if has_hb3:
                hb3r = cp.tile([128, 8], F32)
                nc.scalar.dma_start(hb3r[:], d_hb3r[:])

            # persistent per-chunk activations; variable chunk sizes:
            # small first chunks shorten pipeline fill, small last chunks
            # shorten the drain tail
            CHS = [4, 4, 4, 4]                # tiles (128 rows) per chunk
            OFF = [sum(CHS[:i]) for i in range(len(CHS))]
            NC = len(CHS)
            xTc = [cp.tile([2 * DIM, 128 * nt], BF16, name=f"xTc{c}",
                           tag=f"xT{c}") for c, nt in enumerate(CHS)]
            ATec = [cp.tile([9, 128 * nt], BF16, name=f"ATec{c}",
                            tag=f"AT{c}") for c, nt in enumerate(CHS)]
            Abc = [cp.tile([128, 8 * nt], BF16, name=f"Abc{c}",
                           tag=f"Ab{c}") for c, nt in enumerate(CHS)]
            for c, nt in enumerate(CHS):
                nc.gpsimd.dma_start(
                    ATec[c][8:9, :],
                    d_ones[:, OFF[c] * 128:(OFF[c] + nt) * 128])

            with (
                tc.tile_pool(name="ld", bufs=4) as ld,
                tc.tile_pool(name="psA", bufs=2, space=PSUM) as psA,
                tc.tile_pool(name="gp", bufs=3, space=PSUM) as gp,
                tc.tile_pool(name="cb", bufs=1, space=PSUM) as cb,
                tc.tile_pool(name="tp", bufs=1, space=PSUM) as tp,
                tc.tile_pool(name="sb", bufs=2) as sb,
                tc.tile_pool(name="hb", bufs=6) as hbp,
                tc.tile_pool(name="ob", bufs=2) as obp,
            ):
                zts = {}

                def prefetch(c):
                    if c >= NC or c in zts:
                        return
                    nt = CHS[c]
                    w = 128 * nt
                    sl = slice(OFF[c] * 128, OFF[c] * 128 + w)
                    zt = ld.tile([MLPS, w], BF16, tag="zt")
                    nc.sync.dma_start(zt[:], d_zT[:, sl])
                    nc.sync.dma_start(xTc[c][:], d_xT[:, sl])
                    zts[c] = zt

                def phaseA(c):
                    nt = CHS[c]
                    w = 128 * nt
                    prefetch(c)
                    zt = zts.pop(c)
                    if c == 0:
                        # big head weights: after chunk-0 inputs, needed by
                        # S1(0)/S2(0) a few microseconds later
                        nc.sync.dma_start(w1r[:], d_w1r[:])
                        nc.sync.dma_start(w2r[:], d_w2r[:])
                    # inputs of the NEXT chunk go in flight now so its first
                    # hypernet matmul is never DMA-gated
                    prefetch(c + 1)
                    p0 = psA.tile([H, w], F32, tag="hp")
                    nc.tensor.matmul(p0[:], w0T, zt[:])
                    h0 = ld.tile([H, w], BF16, tag="h")
                    nc.scalar.activation(h0[:], p0[:], TANH, bias=hb0)
                    p1 = psA.tile([H, w], F32, tag="hp")
                    nc.tensor.matmul(p1[:], w1T, h0[:])
                    h1 = ld.tile([H, w], BF16, tag="h")
                    nc.scalar.activation(h1[:], p1[:], TANH, bias=hb1)
                    p2 = psA.tile([H, w], F32, tag="hp")
                    nc.tensor.matmul(p2[:], w2T, h1[:])
                    h2 = ld.tile([H, w], BF16, tag="h")
                    nc.scalar.activation(h2[:], p2[:], TANH, bias=hb2)
                    pAT = psA.tile([8, w], F32, tag="hp")
                    nc.tensor.matmul(pAT[:], w3T, h2[:])
                    nc.scalar.activation(ATec[c][0:8, :], pAT[:], TANH,
                                         bias=hb3)
                    pAC = psA.tile([128, 8 * nt], F32, tag="hp")
                    for j4 in range(nt):
                        nc.tensor.matmul(pAC[:, j4 * 8:(j4 + 1) * 8],
                                         h2[:, j4 * 128:(j4 + 1) * 128], w3T)
                        if has_hb3:
                            nc.vector.tensor_add(
                                pAC[:, j4 * 8:(j4 + 1) * 8],
                                pAC[:, j4 * 8:(j4 + 1) * 8], hb3r[:])
                    nc.scalar.activation(Abc[c][:], pAC[:], TANH)

                st = {}   # per-chunk carried state

                def S1(c):
                    """Stage 1 for chunk c: per-tile g1 matmul + m1 mult,
                    then chunk-batched tree/65th/bias -> hpC [128,nt,65]."""
                    nt = CHS[c]
                    ATc, AbC = ATec[c], Abc[c]
                    # cc bank: c1p (nt x 65) then g1x (nt x 8)
                    gxo = 65 * nt
                    ccC = cb.tile([128, 73 * nt], F32, tag="cc1")
                    # all xT-dependent matmuls first: a still-running
                    # phaseA(c) must not stall later g1s on the in-order PE
                    g1s = []
                    for r in range(nt):
                        rsl = slice(r * 128, (r + 1) * 128)
                        xTj = xTc[c][:, rsl]
                        g1 = gp.tile([128, 8 * DIM], F32, tag="g")
                        nc.tensor.matmul(g1[:], xTj, w1r[:])
                        nc.tensor.matmul(
                            ccC[:, gxo + r * 8:gxo + r * 8 + 8], xTj, w1x)
                        g1s.append(g1)
                    scanC = sb.tile([128, nt, 513], F32, tag="m1")
                    nc.gpsimd.memset(scanC[:, :, 0:1], 0.0)
                    for r in range(nt):
                        rsl = slice(r * 128, (r + 1) * 128)
                        ATj = ATc[:, rsl]
                        nc.tensor.matmul(ccC[:, r * 65:r * 65 + 65], ATj,
                                         hb1w, start=True, stop=not has_hbw1)
                        if has_hbw1:
                            nc.tensor.matmul(ccC[:, r * 65:r * 65 + 65],
                                             xTc[c][:, rsl], hbw1[:],
                                             start=False, stop=True)
                        Abj = AbC[:, r * 8:(r + 1) * 8]
                        nc.vector._custom_dve(
                            MULSCAN,
                            out=scanC[:, r, 1:513],
                            in0=g1s[r][:].rearrange("p (h k) -> p h k", k=8),
                            in1=Abj.unsqueeze(1).broadcast_to([128, DIM, 8]))
                    # per-sample k-sums = running-sum differences at segment
                    # boundaries, one strided subtract for the whole chunk
                    hpC = hbp.tile([128, nt, MLPS], BF16, tag="hpC")
                    nc.vector.tensor_tensor(
                        hpC[:, :, 0:DIM], scanC[:, :, 8::8],
                        scanC[:, :, 0:505:8], AluOpType.subtract)
                    # 65th hidden unit, all nt tiles at once
                    m65 = hbp.tile([128, nt, 8], BF16, tag="m65")
                    nc.vector.tensor_tensor(
                        m65[:],
                        ccC[:, gxo:gxo + 8 * nt]
                        .rearrange("p (t k) -> p t k", k=8),
                        AbC[:].rearrange("p (t k) -> p t k", k=8), mult)
                    with nc.allow_low_precision(reason="8-term bf16 sum"):
                        nc.vector.tensor_reduce(
                            hpC[:, :, DIM:MLPS], m65[:],
                            mybir.AxisListType.X, add)
                    # bias: hpC += c1p (A @ hb1w [+ x @ hbw1])
                    nc.vector.tensor_add(
                        hpC[:].rearrange("p t h -> p (t h)"),
                        hpC[:].rearrange("p t h -> p (t h)"),
                        ccC[:, 0:65 * nt])
                    st[c] = hpC

                def S2(c):
                    """Stage 2 for chunk c: per-tile transpose+tanh+g2+m2,
                    then chunk-batched tree + bias -> out DMA."""
                    nt = CHS[c]
                    hpC = st.pop(c)
                    AbC = Abc[c]
                    cc2C = cb.tile([128, 64 * nt], F32, tag="cc2")
                    scan2C = sb.tile([128, nt, 513], F32, tag="m2")
                    nc.gpsimd.memset(scan2C[:, :, 0:1], 0.0)
                    for r in range(nt):
                        rsl = slice(r * 128, (r + 1) * 128)
                        ATj = ATec[c][:, rsl]
                        nc.tensor.matmul(cc2C[:, r * 64:r * 64 + 64], ATj,
                                         w3re, start=True, stop=not has_hbw2)
                        tpp = tp.tile([MLPS, 128], BF16, tag="tp")
                        nc.tensor.transpose(tpp[:], hpC[:, r, :], ident)
                        hidT = hbp.tile([MLPS, 128], BF16, tag="hidT")
                        nc.scalar.activation(hidT[:], tpp[:], TANH)
                        g2 = gp.tile([128, 8 * DIM], F32, tag="g")
                        nc.tensor.matmul(g2[:], hidT[:], w2r[:])
                        if has_hbw2:
                            nc.tensor.matmul(cc2C[:, r * 64:r * 64 + 64],
                                             hidT[:], hbw2[:],
                                             start=False, stop=True)
                        Abj = AbC[:, r * 8:(r + 1) * 8]
                        nc.vector._custom_dve(
                            MULSCAN,
                            out=scan2C[:, r, 1:513],
                            in0=g2[:].rearrange("p (o k) -> p o k", k=8),
                            in1=Abj.unsqueeze(1).broadcast_to([128, DIM, 8]))
                    o_g = obp.tile([128, nt * DIM], F32, name=f"og{c}",
                                   tag="og")
                    nc.vector.tensor_tensor(
                        o_g[:].rearrange("p (t o) -> p t o", t=nt),
                        scan2C[:, :, 8::8], scan2C[:, :, 0:505:8],
                        AluOpType.subtract)
                    nc.vector.tensor_add(o_g[:], o_g[:], cc2C[:])
                    dst = d_out[OFF[c] * 128:(OFF[c] + nt) * 128,
                                :].rearrange("(t b) o -> b t o", t=nt)
                    nc.sync.dma_start(
                        dst, o_g[:].rearrange("p (t o) -> p t o", t=nt))

                # chunk-level software pipeline, depth 2. phaseA(c+1) is
                # emitted AFTER S1(c)/S2(c-1) so its tanh-latency chain never
                # blocks the g1/g2 matmuls already runnable on the in-order
                # PE queue (DVE work on chunk c covers phaseA(c+1) latency).
                phaseA(0)
                for c in range(NC):
                    S1(c)
                    if c >= 1:
                        S2(c - 1)
                    if c + 1 < NC:
                        phaseA(c + 1)
                S2(NC - 1)

    nc.compile()
    return nc


def _prep(inputs):
    f = lambda name: np.ascontiguousarray(
        np.asarray(inputs[name], dtype=np.float32))
    t = float(np.asarray(inputs["t"]))
    y, freqs = f("y"), f("freqs")
    hw0, hb0 = f("hw0"), f("hb0")
    hw1, hb1 = f("hw1"), f("hb1")
    hw2, hb2 = f("hw2"), f("hb2")
    hw3, hb3 = f("hw3"), f("hb3")
    head_w, head_b = f("head_w"), f("head_b")

    zT = np.empty((MLPS, B), np.float32)
    zT[0, :] = t - 0.5
    zT[1:, :] = freqs.T
    y64 = y[:, :DIM].astype(np.float64)
    xT = np.empty((2 * DIM, B), BF16NP)
    xT[:DIM, :] = np.cos(y64).T.astype(BF16NP)
    xT[DIM:, :] = np.sin(y64).T.astype(BF16NP)

    C = np.ascontiguousarray
    b16 = lambda a: np.asarray(a, dtype=BF16NP)
    w1t = head_w[:O1].reshape(MLPS, 2 * DIM, 8)
    w1r = C(b16(w1t[:DIM].transpose(1, 0, 2).reshape(2 * DIM, 8 * DIM)))
    w2r = C(b16(head_w[O2:O3].reshape(DIM, MLPS, 8)
                .transpose(1, 0, 2).reshape(MLPS, 8 * DIM)))
    hbw1 = C(head_b[:O1].reshape(MLPS, 2 * DIM).T)
    hbw2 = C(head_b[O2:O3].reshape(DIM, MLPS).T)

    wpack = np.zeros((128, WPACK_W), BF16NP)
    wpack[0:MLPS, WO_W0T:WO_W0T + H] = b16(hw0.T)
    wpack[0:H, WO_W1T:WO_W1T + H] = b16(hw1.T)
    wpack[0:H, WO_W2T:WO_W2T + H] = b16(hw2.T)
    wpack[0:H, WO_W3T:WO_W3T + 8] = b16(hw3.T)
    wpack[0:9, WO_HB1W:WO_HB1W + MLPS] = b16(
        np.concatenate([head_w[O1:O2].T, head_b[None, O1:O2]], axis=0))
    wpack[0:9, WO_W3RE:WO_W3RE + DIM] = b16(
        np.concatenate([head_w[O3:].T, head_b[None, O3:]], axis=0))
    wpack[0:2 * DIM, WO_W1X:WO_W1X + 8] = b16(w1t[DIM])
    wpack[0:128, WO_ID:WO_ID + 128] = np.eye(128, dtype=BF16NP)

    bpack = np.zeros((H, 4), np.float32)
    bpack[:, 0], bpack[:, 1], bpack[:, 2] = hb0, hb1, hb2
    bpack[0:8, 3] = hb3

    has_hbw1 = bool(np.any(hbw1))
    has_hbw2 = bool(np.any(hbw2))
    has_hb3 = bool(np.any(hb3))

    shared = {"wpack": wpack, "bpack": bpack, "w1r": w1r, "w2r": w2r,
              "ones": np.ones((1, BS), BF16NP)}
    if has_hbw1:
        shared["hbw1T"] = C(b16(hbw1))
    if has_hbw2:
        shared["hbw2T"] = C(b16(hbw2))
    if has_hb3:
        shared["hb3r"] = C(np.tile(hb3[None, :], (128, 1)))

    zTb = b16(zT)
    in_maps = []
    for c in range(N_CORES):
        sl = slice(c * BS, (c + 1) * BS)
        in_maps.append({
            **shared,
            "zT": C(zTb[:, sl]),
            "xT": C(xT[:, sl]),
        })
    return in_maps, (has_hbw1, has_hbw2, has_hb3)


def _run(inputs, trace=False):
    in_maps, flags = _prep(inputs)
    if flags not in _CACHE:
        _CACHE[flags] = build_bass(*flags)
    nc = _CACHE[flags]
    res = run_bass_kernel_spmd(nc, in_maps, core_ids=list(range(N_CORES)),
                               trace=trace)
    out = np.concatenate([r["out"] for r in res.results], axis=0)
    return out, res


def kernel(**inputs) -> np.ndarray:
    out, _ = _run(inputs)
    return out


if __name__ == "__main__":
    rng = np.random.default_rng(0)
    demo = {
        "t": np.float32(0.3),
        "y": rng.standard_normal((B, 2 * DIM), dtype=np.float32),
        "freqs": rng.random((B, DIM), dtype=np.float32),
        "hw0": rng.standard_normal((H, 1 + DIM), dtype=np.float32) * 0.05,
        "hb0": np.zeros(H, np.float32),
        "hw1": rng.standard_normal((H, H), dtype=np.float32) * 0.05,
        "hb1": np.zeros(H, np.float32),
        "hw2": rng.standard_normal((H, H), dtype=np.float32) * 0.05,
        "hb2": np.zeros(H, np.float32),
        "hw3": rng.standard_normal((8, H), dtype=np.float32) * 0.05,
        "hb3": np.zeros(8, np.float32),
        "head_w": rng.standard_normal((P, 8), dtype=np.float32) * 0.05,
        "head_b": np.zeros(P, np.float32),
    }
    out = kernel(**demo)
    print("out", out.shape, out.dtype, float(np.abs(out).max()))
